# revision 8
# baseline (speedup 1.0000x reference)
"""Extract-last-valid-token kernel for Trainium2 (Bass/Tile), 8-core SPMD.

Computation (per batch row b):
    idx_b = max(sum(attention_mask[b]) - 1, 0)
    out[b] = decoder_outputs[b, idx_b, :]

The reference implements this as a one-hot multiply-reduce over the full
[B, S, H] tensor (256 MiB of reads).  Here each core instead reads only its
mask shard (64 KiB), computes the per-row index on-chip, and gathers the 4
needed rows (2 KiB each) with an indirect DMA — the memory-optimal algorithm.

Sharding: pure data-parallel over the batch dim (B=32 -> 4 rows per core),
no cross-core communication.
"""

import os
import sys
from contextlib import ExitStack

import numpy as np

for _p in ("/opt/trn_rl_repo",):
    if os.path.isdir(_p) and _p not in sys.path:
        sys.path.insert(0, _p)

B, S, H = 32, 4096, 512
N_CORES = 8
BS = B // N_CORES          # batch rows per core
PCHUNK = 32                # partitions used for the mask layout
FCHUNK = S // PCHUNK       # 128 contiguous elements (512 B) per DMA run

_nc_cache = None


def _build_nc():
    """Build the single-core Bass program (same program runs on all 8 cores)."""
    import concourse.bass as bass
    import concourse.tile as tile
    from concourse import bacc, mybir

    nc = bacc.Bacc("TRN2", target_bir_lowering=False, debug=False)

    do = nc.dram_tensor(
        "decoder_outputs", [BS, S, H], mybir.dt.float32, kind="ExternalInput"
    ).ap()
    am = nc.dram_tensor(
        "attention_mask", [BS, S], mybir.dt.int32, kind="ExternalInput"
    ).ap()
    out = nc.dram_tensor(
        "out", [BS, H], mybir.dt.float32, kind="ExternalOutput"
    ).ap()

    with tile.TileContext(nc) as tc, ExitStack() as ctx:
        pool = ctx.enter_context(tc.tile_pool(name="p", bufs=1))
        psum = ctx.enter_context(tc.tile_pool(name="ps", bufs=1, space="PSUM"))

        # CoreV3 engine instructions carry a single sync-wait slot, so every
        # instruction below is arranged to depend on at most one semaphore
        # (same-sem waits coalesce; the engine's observed tick covers earlier
        # producers on an already-waited sem).  The whole index pipeline lives
        # on the free axis of partition 0 so constants can be built with
        # free-offset DVE memsets.
        ones = pool.tile([PCHUNK, 1], mybir.dt.float32)
        nc.vector.memset(ones[:], 1.0)
        one1 = pool.tile([1, 1], mybir.dt.float32)
        nc.vector.memset(one1[:], 1.0)
        offs = pool.tile([1, BS], mybir.dt.float32)
        for b in range(BS):
            nc.vector.memset(offs[:, b : b + 1], float(b * S))

        # Mask shard [BS, S] laid out as [PCHUNK, BS, FCHUNK]: partition p
        # holds, for each row b, the contiguous 512 B run b*S + p*FCHUNK.
        mask_i = pool.tile([PCHUNK, BS, FCHUNK], mybir.dt.int32)
        nc.sync.dma_start(
            out=mask_i[:], in_=am.rearrange("b (p f) -> p b f", p=PCHUNK)
        )

        # Segmented reduce over the free dim: [PCHUNK, BS, FCHUNK] -> [PCHUNK, BS]
        partial_i = pool.tile([PCHUNK, BS], mybir.dt.int32)
        with nc.allow_low_precision(reason="int32 accumulation of 0/1 mask is exact"):
            nc.vector.reduce_sum(
                out=partial_i[:], in_=mask_i[:], axis=mybir.AxisListType.X
            )
        partial_f = pool.tile([PCHUNK, BS], mybir.dt.float32)
        nc.vector.tensor_copy(out=partial_f[:], in_=partial_i[:])

        # Cross-partition sum via matmul: sums[0, b] = sum_p partial_f[p, b].
        sums = psum.tile([1, BS], mybir.dt.float32)
        nc.tensor.matmul(
            out=sums[:], lhsT=ones[:], rhs=partial_f[:], start=True, stop=True
        )

        # idx = max(sum - 1, 0), still f32 (exact: values <= 4095)
        idxf = pool.tile([1, BS], mybir.dt.float32)
        nc.vector.tensor_scalar(
            out=idxf[:],
            in0=sums[:],
            scalar1=-1.0,
            scalar2=0.0,
            op0=mybir.AluOpType.add,
            op1=mybir.AluOpType.max,
        )

        # Global row index into the flattened [BS*S, H] shard: b*S + idx_b.
        idxg_f = pool.tile([1, BS], mybir.dt.float32)
        nc.vector.tensor_tensor(
            out=idxg_f[:], in0=idxf[:], in1=offs[:], op=mybir.AluOpType.add
        )

        # The DGE reads gather indices one-per-partition, so transpose the
        # [1, BS] index row to [BS, 1] with a K=1 matmul, converting to int32
        # on the PSUM->SBUF copy.
        idxg_t = psum.tile([BS, 1], mybir.dt.float32)
        nc.tensor.matmul(
            out=idxg_t[:], lhsT=idxg_f[:], rhs=one1[:], start=True, stop=True
        )
        idxg = pool.tile([BS, 1], mybir.dt.int32)
        nc.vector.tensor_copy(out=idxg[:], in_=idxg_t[:])

        # Gather the BS selected rows (H floats each) from DRAM.
        rows = pool.tile([BS, H], mybir.dt.float32)
        nc.gpsimd.indirect_dma_start(
            out=rows[:],
            out_offset=None,
            in_=do.rearrange("b s h -> (b s) h"),
            in_offset=bass.IndirectOffsetOnAxis(ap=idxg[:, :1], axis=0),
        )

        nc.sync.dma_start(out=out[:], in_=rows[:])

    nc.compile()
    return nc


def build_nc():
    global _nc_cache
    if _nc_cache is None:
        _nc_cache = _build_nc()
    return _nc_cache


def kernel(decoder_outputs, attention_mask):
    from concourse.bass_utils import run_bass_kernel_spmd

    decoder_outputs = np.ascontiguousarray(
        np.asarray(decoder_outputs, dtype=np.float32)
    )
    attention_mask = np.ascontiguousarray(np.asarray(attention_mask, dtype=np.int32))
    assert decoder_outputs.shape == (B, S, H)
    assert attention_mask.shape == (B, S)

    nc = build_nc()
    in_maps = [
        {
            "decoder_outputs": decoder_outputs[i * BS : (i + 1) * BS],
            "attention_mask": attention_mask[i * BS : (i + 1) * BS],
        }
        for i in range(N_CORES)
    ]
    res = run_bass_kernel_spmd(nc, in_maps, list(range(N_CORES)))
    return np.concatenate(
        [res.results[i]["out"] for i in range(N_CORES)], axis=0
    ).astype(np.float32)
